# revision 5
# baseline (speedup 1.0000x reference)
"""HQQ-compatible 4-bit quantized linear layer on 8 Trainium2 NeuronCores.

Problem: y = x @ W.T + bias where W = ((unpack4(W_q) - zero) * scale).reshape(8192, 8192)
  x: (64, 8192) f32; W_q: (32, 1048576) int32 (bytes, two nibbles packed);
  scale/zero: (1, 1048576) f32; bias: (8192,) f32.

Math per output element (OUT=IN=8192, GS=64, NG=2**20):
  W[o, i] = (Wu[gs, ng] - zero[ng]) * scale[ng],  gs = o // 128, ng = (o % 128)*8192 + i
  Wu[r, ng] = W_q[r, ng] >> 4 (r < 32) | W_q[r-32, ng] & 0xF (r >= 32).

Sharding (tensor-parallel over output features, by ng blocks):
  core m owns ng in [m*131072, (m+1)*131072)  <=>  (o % 128) in [m*16, m*16+16).
  core m computes the 1024 outputs o = gs*128 + m*16 + b (gs in [0,64), b in [0,16)).

Per-core device pipeline (linearity: y = sum x*sc*Wu - sum x*(sc*zero) + bias):
  - host splits W_q bytes into hi/lo nibble u8 arrays (bit repacking only),
    flat per-core layout [128, KT*512] (partition-contiguous, chunk-sliceable)
  - hi: HWDGE on the sync ring (dedicated to the 4.2MB nibble stream),
    ScalarE activation-copy casts u8 -> bf16
  - lo: SWDGE (gpsimd ring) casts u8 -> bf16 in-flight
  - consts (x, scale, scale*zero) ride the gpsimd SWDGE ring AHEAD of the lo
    stream; bias rides it at the tail (only needed at the epilogue).  This
    keeps the ScalarE queue free so the first cast starts as soon as the
    first hi chunk lands (the old layout spent ~4us of ScalarE queue time
    dispatching const DMAs before the first ACTIVATE).
  - VectorE: one tensor_tensor mult per (nibble, 8-k pair group): bf16 nibbles
    times scale broadcast over r (2x DVE mode; b-minor unit stride)
  - TensorE: per k two N=512 matmuls (hi|lo) + one N=16 matmul (sc*zero term),
    all accumulating over the 64 k-tiles in PSUM
  - epilogue: tmp = psC_bc - bias (one TT), y = psW - tmp (one TT), DMA out
"""

import ml_dtypes
import numpy as np

OUT = 8192
IN = 8192
GS = 64
NG = OUT * IN // GS  # 1048576
B = 64
NCORES = 8
NGC = NG // NCORES   # 131072 groups per core
BB = 16              # width of the (o % 128) block per core
KT = IN // 128       # 64 in-tiles of 128
CK = 4               # k-tiles per chunk
NCH = KT // CK       # 16 chunks

_CACHE = {}


def _build_nc():
    import concourse.bacc as bacc
    import concourse.mybir as mybir
    import concourse.tile as tile
    from concourse.alu_op_type import AluOpType

    f16 = mybir.dt.bfloat16
    f32 = mybir.dt.float32
    u8 = mybir.dt.uint8

    nc = bacc.Bacc(None, target_bir_lowering=False, debug=False)

    xt_d = nc.dram_tensor("xt", [128, KT * B], f16, kind="ExternalInput")
    hi_d = nc.dram_tensor("hi", [128, KT * 512], u8, kind="ExternalInput")
    lo_d = nc.dram_tensor("lo", [128, KT * 512], u8, kind="ExternalInput")
    sc_d = nc.dram_tensor("sc", [128, KT * BB], f16, kind="ExternalInput")
    sz_d = nc.dram_tensor("sz", [128, KT * BB], f16, kind="ExternalInput")
    bs_d = nc.dram_tensor("bs", [1, 1024], f32, kind="ExternalInput")
    y_d = nc.dram_tensor("y", [B, 1024], f32, kind="ExternalOutput")

    with tile.TileContext(nc) as tc:
        with (
            tc.tile_pool(name="const", bufs=1) as cpool,
            tc.tile_pool(name="wq", bufs=6) as wqpool,
            tc.tile_pool(name="nibhi", bufs=3) as hipool,
            tc.tile_pool(name="niblo", bufs=3) as lopool,
            tc.tile_pool(name="ws", bufs=4) as wspool,
            tc.tile_pool(name="psum", bufs=1, space="PSUM") as pspool,
            tc.tile_pool(name="outp", bufs=1) as opool,
        ):
            # consts on the scalar HWDGE ring (fast first-byte; the SWDGE ring
            # delivers several us late under load).  The dispatches overlap the
            # wait for the first hi chunk, so ScalarE loses no ACT time.
            sc_sb = cpool.tile([128, KT * BB], f16)
            nc.scalar.dma_start(out=sc_sb[:], in_=sc_d[:])
            sz_sb = cpool.tile([128, KT * BB], f16)
            nc.scalar.dma_start(out=sz_sb[:], in_=sz_d[:])
            xt_sb = cpool.tile([128, KT * B], f16)
            nc.scalar.dma_start(out=xt_sb[:], in_=xt_d[:])
            bias_sb = cpool.tile([B, 1024], f32)

            psW = pspool.tile([B, 1024], f32)   # (hi | lo) accumulated
            psC = pspool.tile([B, BB], f32)     # zero-term

            # Zero-term matmuls all up-front: they only need xt+sz (land ~8us),
            # keep the PE busy while the nibble pipeline fills (HAM warm-up),
            # and finish the psC accumulation long before the epilogue.
            for k in range(KT):
                nc.tensor.matmul(
                    psC[:], xt_sb[:, k * B : (k + 1) * B],
                    sz_sb[:, k * BB : (k + 1) * BB],
                    start=(k == 0), stop=(k == KT - 1),
                )

            PK = 2 * CK          # k-tiles per TT/matmul pair-group
            cw = CK * 512
            for pg in range(NCH // 2):
                k0 = pg * PK
                hi_f = hipool.tile([128, PK * 512], f16, tag="hi_f")
                lo_f = lopool.tile([128, PK * 512], f16, tag="lo_f")
                # hi: DMA u8 at CK grain on the sync HWDGE ring, ScalarE casts
                for half in range(2):
                    col0 = (k0 + half * CK) * 512
                    hi_u8 = wqpool.tile([128, cw], u8, tag="hi_u8")
                    nc.sync.dma_start(
                        out=hi_u8[:], in_=hi_d[:, col0 : col0 + cw]
                    )
                    nc.scalar.activation(
                        out=hi_f[:, half * cw : (half + 1) * cw], in_=hi_u8[:],
                        func=mybir.ActivationFunctionType.Copy, scale=1.0,
                    )
                # lo: SWDGE cast-DMA; CK grain on pg0 (faster pipe fill),
                # PK grain (1MB writes) afterwards
                if pg == 0:
                    nc.gpsimd.dma_start(out=lo_f[:, 0:cw], in_=lo_d[:, 0:cw])
                    nc.gpsimd.dma_start(
                        out=lo_f[:, cw : 2 * cw], in_=lo_d[:, cw : 2 * cw]
                    )
                else:
                    nc.gpsimd.dma_start(
                        out=lo_f[:], in_=lo_d[:, k0 * 512 : (k0 + PK) * 512]
                    )

                ws = wspool.tile([128, PK * 1024], f16, tag="ws")
                ws4 = ws[:].rearrange("p (k n) -> p k n", n=1024)

                def sc_view(ka, kb):
                    return (
                        sc_sb[:, ka * BB : kb * BB]
                        .rearrange("p (k b) -> p k b", b=BB)
                        .unsqueeze(2)
                        .broadcast_to((128, kb - ka, 32, BB))
                    )

                # pg0: TTs at CK grain so the first matmuls start sooner
                tt_spans = [(0, CK), (CK, PK)] if pg == 0 else [(0, PK)]
                for (ka, kb) in tt_spans:
                    sc_bc = sc_view(k0 + ka, k0 + kb)
                    nc.vector.tensor_tensor(
                        out=ws4[:, ka:kb, 0:512].rearrange(
                            "p k (r b) -> p k r b", b=BB
                        ),
                        in0=hi_f[:, ka * 512 : kb * 512].rearrange(
                            "p (k r b) -> p k r b", k=kb - ka, b=BB
                        ),
                        in1=sc_bc,
                        op=AluOpType.mult,
                    )
                    nc.vector.tensor_tensor(
                        out=ws4[:, ka:kb, 512:1024].rearrange(
                            "p k (r b) -> p k r b", b=BB
                        ),
                        in0=lo_f[:, ka * 512 : kb * 512].rearrange(
                            "p (k r b) -> p k r b", k=kb - ka, b=BB
                        ),
                        in1=sc_bc,
                        op=AluOpType.mult,
                    )

                for kl in range(PK):
                    k = k0 + kl
                    lhsT = xt_sb[:, k * B : (k + 1) * B]
                    first = k == 0
                    last = k == KT - 1
                    nc.tensor.matmul(
                        psW[:, 0:512], lhsT, ws4[:, kl, 0:512], start=first, stop=last
                    )
                    nc.tensor.matmul(
                        psW[:, 512:1024], lhsT, ws4[:, kl, 512:1024],
                        start=first, stop=last,
                    )

            # bias arrives on the gpsimd ring after the lo stream (it is only
            # needed here, ~35us in)
            nc.gpsimd.dma_start(
                out=bias_sb[:], in_=bs_d[0:1, :].broadcast_to((B, 1024))
            )

            out_sb = opool.tile([B, 1024], f32)
            tmp_sb = opool.tile([B, 1024], f32)
            psC_sb = opool.tile([B, BB], f32)
            nc.scalar.copy(out=psC_sb[:], in_=psC[:])
            # tmp = psC (broadcast over gs) - bias on GpSimd (runs mid-kernel,
            # off the DVE critical path);  y = psW - tmp on DVE (PSUM source)
            nc.gpsimd.tensor_tensor(
                out=tmp_sb[:].rearrange("p (g b) -> p g b", b=BB),
                in0=psC_sb[:].unsqueeze(1).broadcast_to((B, GS, BB)),
                in1=bias_sb[:].rearrange("p (g b) -> p g b", b=BB),
                op=AluOpType.subtract,
            )
            nc.vector.tensor_tensor(
                out=out_sb[:], in0=psW[:], in1=tmp_sb[:], op=AluOpType.subtract
            )
            nc.sync.dma_start(out=y_d[:], in_=out_sb[:])

    nc.compile()
    return nc


def _get_nc():
    if "nc" not in _CACHE:
        _CACHE["nc"] = _build_nc()
    return _CACHE["nc"]


def _prep_inputs(x, W_q, scale, zero, bias):
    """Host-side shard + layout prep (dtype narrowing / bit repack / transposes)."""
    xt = (
        x.T.reshape(KT, 128, B).transpose(1, 0, 2).reshape(128, KT * B)
    ).astype(ml_dtypes.bfloat16)  # (p, (k t))
    wq_u8 = W_q.astype(np.uint8)
    hi_u8 = (wq_u8 >> 4).astype(np.uint8)
    lo_u8 = (wq_u8 & 0xF).astype(np.uint8)
    sz_full = (scale.astype(np.float64) * zero.astype(np.float64)).astype(np.float32)

    def wlayout(arr_m):
        # arr_m: (32, NGC) one core's nibble slice -> [p, (k, r, b)] flat
        a = arr_m.reshape(32, BB, IN)          # (r, b, in)
        a = a.transpose(2, 0, 1)               # (in, r, b): col = r*16+b
        a = a.reshape(KT, 128, 512)            # (k, p, rb)
        a = a.transpose(1, 0, 2)               # (p, k, rb)
        return np.ascontiguousarray(a.reshape(128, KT * 512))

    in_maps = []
    for m in range(NCORES):
        sl = slice(m * NGC, (m + 1) * NGC)
        sc_m = (
            scale[0, sl]
            .reshape(BB, IN)
            .T.reshape(KT, 128, BB)
            .transpose(1, 0, 2)
            .reshape(128, KT * BB)
        ).astype(ml_dtypes.bfloat16)
        sz_m = (
            sz_full[0, sl]
            .reshape(BB, IN)
            .T.reshape(KT, 128, BB)
            .transpose(1, 0, 2)
            .reshape(128, KT * BB)
        ).astype(ml_dtypes.bfloat16)
        # out col c = h*512 + r*16 + b  <->  global out o = (h*32+r)*128 + m*16 + b
        bs_m = (
            bias.reshape(GS, 128)[:, m * BB : (m + 1) * BB]  # (gs, b)
            .reshape(1, 1024)
            .astype(np.float32)
        )
        in_maps.append(
            {
                "xt": xt,
                "hi": wlayout(hi_u8[:, sl]),
                "lo": wlayout(lo_u8[:, sl]),
                "sc": np.ascontiguousarray(sc_m),
                "sz": np.ascontiguousarray(sz_m),
                "bs": bs_m,
            }
        )
    return in_maps


def _gather(results):
    ybig = np.stack([results[m]["y"] for m in range(NCORES)], axis=1)  # (t, m, 1024)
    ybig = ybig.reshape(B, NCORES, 2, 32, BB)  # (t, m, h, r, b)
    return np.ascontiguousarray(
        ybig.transpose(0, 2, 3, 1, 4).reshape(B, OUT)
    )  # o = (h*32+r)*128 + m*16 + b


def run_on_hw(x, W_q, scale, zero, bias, trace=False, **trace_kw):
    """Returns (y_full, BassKernelResults)."""
    from concourse.bass_utils import run_bass_kernel_spmd

    nc = _get_nc()
    in_maps = _prep_inputs(x, W_q, scale, zero, bias)
    res = run_bass_kernel_spmd(
        nc, in_maps, list(range(NCORES)), trace=trace, **trace_kw
    )
    return _gather(res.results), res


def kernel(x, W_q, scale, zero, bias):
    y, _ = run_on_hw(x, W_q, scale, zero, bias, trace=False)
    return y
